# revision 1
# baseline (speedup 1.0000x reference)
"""Trainium2 Bass kernel: 3x chained zentorch_qlinear (M=8192, K=N=4096).

Strategy (8 NeuronCores, data-parallel over M; no collectives):
  - Each core gets 1024 rows of the input and the full weight matrix.
  - Quantized activations are held as *integer-valued bf16* (exact for
    |v| <= 255), transposed layout aqT[k, m], SBUF-resident.
  - Weights are cast int32->bf16 once on device (integer values, exact) and
    written to a DRAM scratch copy; each layer streams stationary tiles
    [k,n] via the XBAR DMA-transpose directly from that natural-layout copy.
  - Matmuls run in yT orientation: psum[n, m] = sum_k WT[k,n] * aqT[k,m],
    so each layer's output psum is already in the transposed layout the
    next layer consumes.
  - Quantize (scale, +bias, round-half-even, saturate) is a single ScalarE
    ACTIVATE with per-partition scale/bias APs and uint8 output, followed
    by one VectorE (x - zp) -> bf16 recenter.
  - Final layer: psum -> (scale+bias) fp32, PE-transposed back to natural
    [m, n] in 128x128 pieces (pipelined one n-block behind the matmuls) and
    written straight to the output with one batched DMA per n-block.
"""

import numpy as np

M, K, N = 8192, 4096, 4096
NCORES = 8
ML = M // NCORES  # 1024 rows per core
NBLK = N // 128   # 32
KBLK = K // 128   # 32
MSLAB = ML // 128  # 8
PC = 1024          # prep chunk width (columns)

_COMPILED = {}


def _build(inv_s: float, zp: float):
    import concourse.bacc as bacc
    import concourse.mybir as mybir
    import concourse.tile as tile
    from concourse.masks import make_identity

    dt = mybir.dt
    AF = mybir.ActivationFunctionType
    Alu = mybir.AluOpType

    nc = bacc.Bacc("TRN2", target_bir_lowering=False, debug=False, num_devices=NCORES)

    x = nc.dram_tensor("x", [ML, K], dt.float32, kind="ExternalInput")
    w = nc.dram_tensor("w", [N, K], dt.int32, kind="ExternalInput")
    # per-output-channel vectors arranged [p, j] with column j = v[j*128:(j+1)*128]
    swq = nc.dram_tensor("swq", [128, NBLK], dt.float32, kind="ExternalInput")
    bq = nc.dram_tensor("bq", [128, NBLK], dt.float32, kind="ExternalInput")
    swo = nc.dram_tensor("swo", [128, NBLK], dt.float32, kind="ExternalInput")
    bo = nc.dram_tensor("bo", [128, NBLK], dt.float32, kind="ExternalInput")
    y = nc.dram_tensor("y", [ML, N], dt.float32, kind="ExternalOutput")

    with tile.TileContext(nc) as tc:
        with (
            tc.tile_pool(name="consts", bufs=1) as cpool,
            tc.tile_pool(name="aq", bufs=1) as aqpool,
            tc.tile_pool(name="stat", bufs=3) as statpool,
            tc.tile_pool(name="dram", bufs=1, space="DRAM") as dpool,
            tc.tile_pool(name="mm", bufs=2, space="PSUM") as mmpool,
            tc.tile_pool(name="tp", bufs=2, space="PSUM") as tppool,
            tc.tile_pool(name="qstage", bufs=2) as qpool,
        ):
            swq_t = cpool.tile([128, NBLK], dt.float32)
            bq_t = cpool.tile([128, NBLK], dt.float32)
            swo_t = cpool.tile([128, NBLK], dt.float32)
            bo_t = cpool.tile([128, NBLK], dt.float32)
            ident = cpool.tile([128, 128], dt.float32)
            zp_col = cpool.tile([128, 1], dt.float32)
            nc.gpsimd.memset(zp_col[:], zp)
            nc.sync.dma_start(out=swq_t[:], in_=swq[:])
            nc.sync.dma_start(out=bq_t[:], in_=bq[:])
            nc.sync.dma_start(out=swo_t[:], in_=swo[:])
            nc.sync.dma_start(out=bo_t[:], in_=bo[:])
            make_identity(nc, ident[:])

            # activations, transposed: [k within blk, k_blk, m_slab, m within slab]
            aqA = aqpool.tile([128, KBLK, MSLAB, 128], dt.bfloat16, name="aqA")
            aqB = aqpool.tile([128, KBLK, MSLAB, 128], dt.bfloat16, name="aqB")

            wslabs = [
                dpool.tile([128, K], dt.bfloat16, name=f"wslab{j}", tag=f"wslab{j}")
                for j in range(NBLK)
            ]

            nch = K // PC  # chunks per 128-row slab
            kb_per = PC // 128  # k-blocks covered per chunk
            with tc.tile_pool(name="prep", bufs=2) as ppool:
                # ---- X prep: quantize + transpose into aqA
                # (k-half outer so low-k strips finish first and layer 1's
                #  psum accumulation can begin at ~50% of x-prep)
                for h in range(nch):
                    for s in range(MSLAB):
                        xs = ppool.tile([128, PC], dt.float32, name="xs", tag="xs")
                        nc.sync.dma_start(
                            out=xs[:], in_=x[s * 128 : (s + 1) * 128, h * PC : (h + 1) * PC]
                        )
                        qu = ppool.tile([128, PC], dt.uint8, name="qu", tag="qu")
                        nc.scalar.activation(
                            qu[:], xs[:], AF.Identity, bias=zp_col[:, 0:1], scale=inv_s
                        )
                        qb = ppool.tile([128, PC], dt.bfloat16, name="qb", tag="qb")
                        nc.vector.tensor_scalar(qb[:], qu[:], zp, None, Alu.subtract)
                        tst = ppool.tile([128, kb_per, 128], dt.bfloat16, name="tst", tag="tst")
                        nc.sync.dma_start_transpose(out=tst[:], in_=qb[:])
                        nc.vector.tensor_copy(
                            aqA[:, h * kb_per : (h + 1) * kb_per, s, :], tst[:]
                        )

                # ---- W prep: int32 -> bf16 natural-layout DRAM copy
                for j in range(NBLK):
                    for h in range(nch):
                        wi = ppool.tile([128, PC], dt.int32, name="wi", tag="wi")
                        nc.sync.dma_start(
                            out=wi[:], in_=w[j * 128 : (j + 1) * 128, h * PC : (h + 1) * PC]
                        )
                        wb = ppool.tile([128, PC], dt.bfloat16, name="wb", tag="wb")
                        nc.vector.tensor_copy(wb[:], wi[:])
                        nc.sync.dma_start(
                            out=wslabs[j][:, h * PC : (h + 1) * PC], in_=wb[:]
                        )

            # ---- 3 chained qlinear layers (yT orientation)
            y_r = y[:].rearrange("(s p) n -> p s n", p=128)
            pend = []

            def emit_out(j, y3sb):
                piece = qpool.tile([128, MSLAB, 128], dt.float32, name="piece", tag="piece")
                for half in range(2):
                    pst = tppool.tile([128, 512], dt.float32, name="pst", tag="pst")
                    for c in range(4):
                        mb = half * 4 + c
                        nc.tensor.transpose(
                            pst[:, c * 128 : (c + 1) * 128],
                            y3sb[:, mb * 128 : (mb + 1) * 128], ident[:],
                        )
                    nc.vector.tensor_copy(piece[:, half * 4 : (half + 1) * 4, :], pst[:])
                nc.sync.dma_start(out=y_r[:, :, j * 128 : (j + 1) * 128], in_=piece[:])

            for l in range(3):
                IN = aqA if l != 1 else aqB
                OUT = aqB if l == 0 else aqA
                for j in range(NBLK):
                    stat = statpool.tile([128, KBLK, 128], dt.bfloat16, name="stat", tag="stat")
                    nc.sync.dma_start_transpose(out=stat[:], in_=wslabs[j][:])
                    ps = [
                        mmpool.tile([128, 512], dt.float32, name=f"ps{h}", tag=f"ps{h}")
                        for h in range(2)
                    ]
                    for k in range(KBLK):
                        for h in range(2):
                            nc.tensor.matmul(
                                ps[h][:],
                                stat[:, k, :],
                                IN[:, k, 4 * h : 4 * h + 4, :],
                                start=(k == 0),
                                stop=(k == KBLK - 1),
                            )
                    if l < 2:
                        for h in range(2):
                            qh = qpool.tile([128, 512], dt.uint8, name="qh", tag="qh")
                            nc.scalar.activation(
                                qh[:], ps[h][:], AF.Identity,
                                bias=bq_t[:, j : j + 1], scale=swq_t[:, j : j + 1],
                            )
                            nc.vector.tensor_scalar(
                                OUT[:, j, 4 * h : 4 * h + 4, :], qh[:], zp, None, Alu.subtract
                            )
                    else:
                        y3sb = qpool.tile([128, 1024], dt.float32, name="y3sb",
                                          tag="y3sb", bufs=3)
                        for h in range(2):
                            nc.scalar.activation(
                                y3sb[:, h * 512 : (h + 1) * 512], ps[h][:], AF.Identity,
                                bias=bo_t[:, j : j + 1], scale=swo_t[:, j : j + 1],
                            )
                        pend.append((j, y3sb))
                        if len(pend) > 1:
                            emit_out(*pend.pop(0))

            while pend:
                emit_out(*pend.pop(0))

    nc.compile()
    return nc


def kernel(input, weights, biases, input_scales, input_zero_points,
           weight_scales, weight_zero_points, output_dtype=None):
    from concourse.bass_utils import run_bass_kernel_spmd

    input = np.asarray(input, dtype=np.float32)
    weights = np.ascontiguousarray(np.asarray(weights, dtype=np.int32))
    biases = np.asarray(biases, dtype=np.float32)
    s_in = np.float32(np.asarray(input_scales).reshape(-1)[0])
    zp_in = float(np.asarray(input_zero_points).reshape(-1)[0])
    s_w = np.asarray(weight_scales, dtype=np.float32)

    inv_s = float(np.float32(1.0) / s_in)
    key = (inv_s, zp_in)
    if key not in _COMPILED:
        _COMPILED[key] = _build(inv_s, zp_in)
    nc = _COMPILED[key]

    def arrange(v):
        return np.ascontiguousarray(v.reshape(NBLK, 128).T.astype(np.float32))

    swq_v = arrange(s_w)
    bq_v = arrange(biases / s_in + np.float32(zp_in))
    swo_v = arrange(s_w * s_in)
    bo_v = arrange(biases)

    in_maps = []
    for i in range(NCORES):
        in_maps.append({
            "x": np.ascontiguousarray(input[i * ML : (i + 1) * ML]),
            "w": weights,
            "swq": swq_v,
            "bq": bq_v,
            "swo": swo_v,
            "bo": bo_v,
        })

    res = run_bass_kernel_spmd(nc, in_maps, core_ids=list(range(NCORES)))
    out = np.concatenate([res.results[i]["y"] for i in range(NCORES)], axis=0)
    return out.astype(np.float32)


if __name__ == "__main__":
    rng = np.random.default_rng(0)
    inp = {
        "input": rng.normal(size=(M, K)).astype(np.float32),
        "weights": rng.integers(-128, 128, (N, K), dtype=np.int32),
        "biases": (rng.normal(size=(N,)) * 0.1).astype(np.float32),
        "input_scales": np.array([0.05], np.float32),
        "input_zero_points": np.array([128], np.int32),
        "weight_scales": rng.uniform(0.001, 0.01, (N,)).astype(np.float32),
        "weight_zero_points": np.zeros((N,), np.int32),
        "output_dtype": 0,
    }
    out = kernel(**inp)
    print(out.shape, out.dtype, np.abs(out).mean())

